# revision 21
# baseline (speedup 1.0000x reference)
"""NodeClustering (vq_codebook) Trainium2 kernel — v3.

Math (per batch element b, P=16384 points, C=256 channels, K=8 clusters):
  nodes = F_p @ proj_w.T + proj_b
  3 iterations of: sim = l2(nodes) @ l2(centers).T ; assign = argmax;
                   centers = segment_mean(nodes)
  weights = softmax(10 * l2(nodes) @ l2(centers).T)
  out = (weights@centers + F_p) @ refine_w.T + refine_b

Algebraic restructuring (as v1):
  nodes[p] . cn_k    = F_p[p] . G_k + h_k      (G = A @ cn, h = pb . cn)
  segment_sum(nodes) = segment_sum(F_p) @ pwT + counts * pb
  out                = F_p @ rwT + w @ Dm      (Dm = centers @ rwT + rb)
  ||nodes_p||^2      = ||F_p@pwT||^2 + F_p.(2 pwT@pb) + ||pb||^2
                       (cross term rides as an extra rhs column)

Execution notes (measured: PE runs ~1.2 GHz, per-matmul cost ~(91+N)/1.2 ns,
so instruction count and N both matter):
  - fT via PE transposes (not Sync DMA transposes), batched PSUM->SBUF copy.
  - Software-pipelined chunk skew so the in-order PE queue never waits on
    DVE/ACT results.
  - S-matmuls col-tiled 4x: four chunks' onehot weights live in distinct
    32-column strips of the PE array and stream concurrently; partials at
    partition 0/32/64/96 are combined by 4 diagonal-identity matmuls.
  - Iteration argmax batched 8 chunks per DVE op; +h via broadcast DVE add.
  - Phase 5: per-4-chunk padded weight transpose (one PE op), row-tiled 4x
    concurrent w@Dm accumulating into each chunk's op_ PSUM (weights
    prenormalized), DmB replicated to partitions 0/32/64/96.

Sharding: pure data parallel, core i <- batch element i (B=8, 8 cores).
"""

import sys
import numpy as np

sys.path.insert(0, "/opt/trn_rl_repo")

import concourse.bass as bass
import concourse.bacc as bacc
import concourse.mybir as mybir
import concourse.tile as tile
from concourse._compat import get_trn_type
from concourse.bass_utils import axon_active
from concourse.masks import make_identity
from concourse.bass_utils import run_bass_kernel_spmd

P = 16384
C = 256
NK = 8
NUM_ITERS = 3
EPS = 1e-12
N_CORES = 8
NCHUNK = P // 128
U_IN = 8    # chunks per input DMA (1 MiB)
U_OUT = 4   # chunks per output DMA / phase-5 group
GSZ = 8     # chunks per argmax batch in iterations
SKW_T = 2   # transpose lag behind cast (phase 1)
SKW_Y = 6   # y matmul lag behind transpose copy (phase 1)
SKW_S = 10  # S matmul lag behind argmax (phase 1)

F32 = mybir.dt.float32
BF16 = mybir.dt.bfloat16
AF = mybir.ActivationFunctionType
ALU = mybir.AluOpType

COLTILE_S = False   # bisect: col-tiled S matmuls
ROWTILE_W = True    # bisect: row-tiled phase-5 wDm


def build_bass(p=P):
    nchunk = p // 128
    idx = list(np.linspace(0, p - 1, NK).astype(np.int64))
    nc = bacc.Bacc(
        get_trn_type() or "TRN2",
        target_bir_lowering=False,
        debug=not axon_active(),
        num_devices=N_CORES,
    )

    fp = nc.dram_tensor("fp", [p, C], F32, kind="ExternalInput")
    pw = nc.dram_tensor("pw", [C, C], F32, kind="ExternalInput")
    pb = nc.dram_tensor("pb", [C], F32, kind="ExternalInput")
    rw = nc.dram_tensor("rw", [C, C], F32, kind="ExternalInput")
    rb = nc.dram_tensor("rb", [C], F32, kind="ExternalInput")
    out = nc.dram_tensor("out", [p, C], F32, kind="ExternalOutput")

    fp_v = fp[:].rearrange("(n p) c -> p n c", p=128)
    out_v = out[:].rearrange("(n p) c -> p n c", p=128)

    with tile.TileContext(nc) as tc:
        with (
            tc.tile_pool(name="res", bufs=1) as res,        # persistent tiles
            tc.tile_pool(name="nat", bufs=3) as natp,       # streamed F_p chunks
            tc.tile_pool(name="outp", bufs=3) as outp,      # output staging
            tc.tile_pool(name="sml", bufs=12) as sml,       # per-chunk small tiles
            tc.tile_pool(name="grp", bufs=2) as grp,        # group-batched tiles
            tc.tile_pool(name="it", bufs=1) as itp,         # per-iteration small tiles
        ):
            # ---------------- constants + weights ----------------
            ident = res.tile([128, 128], F32)
            make_identity(nc, ident)
            ident_b = res.tile([128, 128], BF16)
            make_identity(nc, ident_b)
            ones_row = res.tile([1, 128], F32)
            nc.vector.memset(ones_row, 1.0)
            ones_rowB = res.tile([1, 128], BF16)
            nc.vector.memset(ones_rowB, 1.0)

            pw_n = res.tile([128, 2, C], F32)    # proj_w rows (c' partition)
            nc.sync.dma_start(out=pw_n, in_=pw[:].rearrange("(h p) c -> p h c", p=128))
            rw_n = res.tile([128, 2, C], F32)
            nc.sync.dma_start(out=rw_n, in_=rw[:].rearrange("(h p) c -> p h c", p=128))
            pb_col = res.tile([128, 2], F32)
            nc.sync.dma_start(out=pb_col, in_=pb[:].rearrange("(h p) -> p h", p=128))
            pb_row = res.tile([1, C], F32)
            nc.sync.dma_start(out=pb_row, in_=pb[:].unsqueeze(0))
            rb_row = res.tile([1, C], F32)
            nc.sync.dma_start(out=rb_row, in_=rb[:].unsqueeze(0))

            # residents
            natB = res.tile([128, nchunk, C + 1], BF16)  # F_p natural + ones col
            nc.vector.memset(natB[:, :, C:C + 1], 1.0)
            fTB = res.tile([128, 2, p], BF16)   # F_p.T halves (c partition)
            SQ = res.tile([128, nchunk], F32)   # ||F@pwT||^2 per point
            CR = res.tile([128, nchunk], F32)   # cross term per point
            inv10 = res.tile([128, nchunk], F32)
            pwT = res.tile([128, 2, C], F32)
            rwT = res.tile([128, 2, C], F32)
            # [pwT | v2 | G1] bf16 rhs for the phase-1 matmul
            projrhsB = res.tile([128, 2, C + 1 + NK], BF16)
            hrow1B = res.tile([1, NK], BF16)    # iter-1 h row (ones matmul rhs)
            # [rwT | Gf] bf16 rhs for phase 5
            catB = res.tile([128, 2, C + NK], BF16)
            hFB = res.tile([1, NK], BF16)       # ones-row rhs for phase-5 sims
            DmRepB = res.tile([128, C], BF16)   # Dm replicated at 0/32/64/96
            h128 = res.tile([128, NK], F32)     # iteration h, partition-bcast
            h128g = res.tile([128, GSZ, NK], F32)  # h replicated per group slot
            pbb_col = res.tile([128, 1], F32)   # ||pb||^2 broadcast

            with tc.tile_pool(name="ps_acc", bufs=1, space="PSUM") as ps_acc:

                def make_G(centers_sb, Gdst, hrowB_dst=None, want_h128=False):
                    """centers (8,C) f32 -> l2norm -> bf16 G halves + h outputs."""
                    csq = itp.tile([NK, C], F32, tag="csq")
                    cn2 = itp.tile([NK, 1], F32, tag="cn2")
                    nc.scalar.activation(csq, centers_sb, AF.Square, accum_out=cn2)
                    nc.scalar.sqrt(cn2, cn2)
                    nc.vector.tensor_scalar(cn2, cn2, EPS, None, op0=ALU.max)
                    rin = itp.tile([NK, 1], F32, tag="rin")
                    nc.vector.reciprocal(rin, cn2)
                    cn = itp.tile([NK, C], F32, tag="cn")
                    nc.vector.tensor_scalar_mul(cn, centers_sb, rin)
                    cnT = itp.tile([128, 2, NK], F32, tag="cnT")
                    for h in range(2):
                        tp = ps_acc.tile([128, NK], F32, tag="ips")
                        nc.tensor.transpose(tp, cn[:, h * 128:(h + 1) * 128],
                                            ident[0:NK, 0:NK])
                        nc.vector.tensor_copy(cnT[:, h], tp)
                    # G[c,k] = sum_c' proj_w[c',c] cnT[c',k]
                    for mh in range(2):
                        gp = ps_acc.tile([128, NK], F32, tag="ips")
                        nc.tensor.matmul(gp, pw_n[:, 0, mh * 128:(mh + 1) * 128],
                                         cnT[:, 0], start=True, stop=False)
                        nc.tensor.matmul(gp, pw_n[:, 1, mh * 128:(mh + 1) * 128],
                                         cnT[:, 1], start=False, stop=True)
                        nc.vector.tensor_copy(Gdst(mh), gp)
                    # h_k = pb . cn_k
                    hp = ps_acc.tile([1, NK], F32, tag="ips2")
                    nc.tensor.matmul(hp, pb_col[:, 0:1], cnT[:, 0],
                                     start=True, stop=False)
                    nc.tensor.matmul(hp, pb_col[:, 1:2], cnT[:, 1],
                                     start=False, stop=True)
                    if hrowB_dst is not None:
                        nc.vector.tensor_copy(hrowB_dst, hp)
                    if want_h128:
                        hrow_sb = itp.tile([1, NK], F32, tag="hrow")
                        nc.vector.tensor_copy(hrow_sb, hp)
                        hbp = ps_acc.tile([128, NK], F32, tag="ips2")
                        nc.tensor.matmul(hbp, ones_row, hrow_sb)
                        nc.vector.tensor_copy(h128, hbp)
                        for jj in range(GSZ):
                            nc.vector.tensor_copy(h128g[:, jj, :], h128)

                def combine_S(Sacc, nstrip):
                    """Sum the col-tiled partials (partitions 0/32/..)."""
                    if nstrip == 1:
                        return Sacc[0:NK, :]
                    Ssb = itp.tile([64, C + 1], F32, tag="Ssb")
                    nc.vector.tensor_copy(Ssb, Sacc[0:64, :])
                    S2 = ps_acc.tile([NK, C + 1], F32, tag="ips2")
                    for q in range(nstrip):
                        b = 32 * q
                        nc.tensor.matmul(S2, ident[b:b + NK, b:b + NK],
                                         Ssb[b:b + NK, :],
                                         start=(q == 0), stop=(q == nstrip - 1),
                                         tile_position=(b, 0))
                    return S2

                def update_centers(Sacc, nstrip=1):
                    S2 = combine_S(Sacc, nstrip)
                    cnt = itp.tile([NK, 1], F32, tag="cntsb")
                    nc.vector.tensor_scalar(cnt, S2[:, C:C + 1], 1.0, None,
                                            op0=ALU.max)
                    nc.vector.reciprocal(cnt, cnt)
                    fmean = itp.tile([NK, C], F32, tag="fmean")
                    nc.vector.tensor_scalar_mul(fmean, S2[:, 0:C], cnt)
                    fmT = itp.tile([128, 2, NK], F32, tag="fmT")
                    for h in range(2):
                        tp = ps_acc.tile([128, NK], F32, tag="ips")
                        nc.tensor.transpose(tp, fmean[:, h * 128:(h + 1) * 128],
                                            ident[0:NK, 0:NK])
                        nc.vector.tensor_copy(fmT[:, h], tp)
                    cp = ps_acc.tile([NK, C], F32, tag="ips")
                    nc.tensor.matmul(cp, fmT[:, 0], pwT[:, 0], start=True, stop=False)
                    nc.tensor.matmul(cp, fmT[:, 1], pwT[:, 1], start=False, stop=False)
                    nc.tensor.matmul(cp, ones_row[:, 0:NK], pb_row,
                                     start=False, stop=True)
                    centers = itp.tile([NK, C], F32, tag="centers")
                    nc.vector.tensor_copy(centers, cp)
                    return centers

                # ---------------- phase 1 (+ setup) ----------------
                with (
                    tc.tile_pool(name="ps_tr", bufs=2, space="PSUM") as ps_tr,
                    tc.tile_pool(name="ps_y", bufs=3, space="PSUM") as ps_y,
                ):
                    # pwT/rwT: transposed weights via PE (f32)
                    for dst, src in ((pwT, pw_n), (rwT, rw_n)):
                        for kh in range(2):      # source partition half (c')
                            for mh in range(2):  # source free half (c)
                                tp = ps_acc.tile(
                                    [128, 128], F32,
                                    tag="ips" if (kh + mh) % 2 == 0 else "ips2")
                                nc.tensor.transpose(
                                    tp, src[:, kh, mh * 128:(mh + 1) * 128], ident)
                                if (kh + mh) % 2:
                                    nc.vector.tensor_copy(
                                        dst[:, mh, kh * 128:(kh + 1) * 128], tp)
                                else:
                                    nc.scalar.activation(
                                        dst[:, mh, kh * 128:(kh + 1) * 128], tp,
                                        AF.Copy)
                    for h in range(2):
                        nc.vector.tensor_copy(projrhsB[:, h, 0:C], pwT[:, h])
                        nc.scalar.activation(catB[:, h, 0:C], rwT[:, h], AF.Copy)
                    # v2 = 2 * pwT @ pb, split into column halves
                    for mh in range(2):
                        vp = ps_acc.tile([128, 1], F32, tag="ips")
                        nc.tensor.matmul(vp, pw_n[:, 0, mh * 128:(mh + 1) * 128],
                                         pb_col[:, 0:1], start=True, stop=False)
                        nc.tensor.matmul(vp, pw_n[:, 1, mh * 128:(mh + 1) * 128],
                                         pb_col[:, 1:2], start=False, stop=True)
                        nc.scalar.activation(projrhsB[:, mh, C:C + 1], vp, AF.Copy,
                                             scale=2.0)
                    # pbb = ||pb||^2 broadcast to a [128,1] column
                    pbsq = itp.tile([1, C], F32, tag="pbsq")
                    pbb1 = itp.tile([1, 1], F32, tag="pbb1")
                    nc.scalar.activation(pbsq, pb_row, AF.Square, accum_out=pbb1)
                    pbb_ps = ps_acc.tile([128, 1], F32, tag="ips")
                    nc.tensor.matmul(pbb_ps, ones_row, pbb1)
                    nc.vector.tensor_copy(pbb_col, pbb_ps)

                    # seed rows of F_p for initial centers, gathered from DRAM
                    gatT = itp.tile([NK, C], F32, tag="gatT")
                    for k, g in enumerate(idx):
                        nc.sync.dma_start(out=gatT[k:k + 1, :], in_=fp[g:g + 1, :])
                    gatc = itp.tile([128, 2, NK], F32, tag="gatc")
                    for h in range(2):
                        tp = ps_acc.tile([128, NK], F32, tag="ips")
                        nc.tensor.transpose(tp, gatT[:, h * 128:(h + 1) * 128],
                                            ident[0:NK, 0:NK])
                        nc.vector.tensor_copy(gatc[:, h], tp)
                    c0 = ps_acc.tile([NK, C], F32, tag="ips")
                    nc.tensor.matmul(c0, gatc[:, 0], pwT[:, 0], start=True, stop=False)
                    nc.tensor.matmul(c0, gatc[:, 1], pwT[:, 1], start=False, stop=False)
                    nc.tensor.matmul(c0, ones_row[:, 0:NK], pb_row,
                                     start=False, stop=True)
                    centers = itp.tile([NK, C], F32, tag="centers")
                    nc.vector.tensor_copy(centers, c0)
                    make_G(centers, lambda h: projrhsB[:, h, C + 1:C + 1 + NK],
                           hrowB_dst=hrow1B)

                    # ---- streamed loop ----
                    Sacc = ps_acc.tile([128, C + 1], F32, tag="S")
                    state = {"nt": None,
                             "sqt": grp.tile([128, GSZ, C], BF16, tag="sqt",
                                             name="sqt")}

                    def st_in(ci):
                        if ci % U_IN == 0:
                            state["nt"] = natp.tile([128, U_IN, C], F32, tag="nat",
                                                    name="nt")
                            gi = ci // U_IN
                            nc.sync.dma_start(
                                out=state["nt"],
                                in_=fp_v[:, gi * U_IN:(gi + 1) * U_IN, :])
                        j = ci % U_IN
                        if ci % 4 == 1:
                            nc.vector.tensor_copy(natB[:, ci, 0:C],
                                                  state["nt"][:, j, :])
                        else:
                            nc.gpsimd.tensor_copy(natB[:, ci, 0:C],
                                                  state["nt"][:, j, :])

                    def st_tr(ci):
                        sl = slice(ci * 128, (ci + 1) * 128)
                        trt = ps_tr.tile([128, 2, 128], BF16, tag="tr")
                        nc.tensor.transpose(trt[:, 0, :], natB[:, ci, 0:128],
                                            ident_b)
                        nc.tensor.transpose(trt[:, 1, :], natB[:, ci, 128:C],
                                            ident_b)
                        nc.scalar.activation(fTB[:, :, sl], trt, AF.Copy)

                    def st_y(ci):
                        sl = slice(ci * 128, (ci + 1) * 128)
                        y = ps_y.tile([128, C + 1 + NK], F32, tag="y")
                        nc.tensor.matmul(y, fTB[:, 0, sl], projrhsB[:, 0],
                                         start=True, stop=False)
                        nc.tensor.matmul(y[:, C + 1:C + 1 + NK], ones_rowB, hrow1B,
                                         start=False, stop=False,
                                         skip_group_check=True)
                        nc.tensor.matmul(y, fTB[:, 1, sl], projrhsB[:, 1],
                                         start=False, stop=True)
                        nc.scalar.activation(state["sqt"][:, ci % GSZ, :],
                                             y[:, 0:C], AF.Square)
                        nc.vector.tensor_copy(CR[:, ci:ci + 1], y[:, C:C + 1])
                        mx = sml.tile([128, 1], F32, tag="mx")
                        nc.vector.tensor_reduce(mx, y[:, C + 1:C + 1 + NK],
                                                axis=mybir.AxisListType.X, op=ALU.max)
                        oh = sml.tile([128, NK], BF16, tag="oh")
                        nc.vector.tensor_scalar(oh, y[:, C + 1:C + 1 + NK],
                                                mx[:, 0:1], None, op0=ALU.is_ge)
                        if ci % GSZ == GSZ - 1:
                            g0 = (ci // GSZ) * GSZ
                            nc.vector.tensor_reduce(
                                SQ[:, g0:g0 + GSZ], state["sqt"],
                                axis=mybir.AxisListType.X, op=ALU.add)
                            state["sqt"] = grp.tile([128, GSZ, C], BF16, tag="sqt",
                                                    name="sqt")
                        return oh

                    def s_burst(c0, ohs):
                        # 4 col-tiled S matmuls: chunk c0+q -> col strip q
                        for q in range(4):
                            cs = c0 + q
                            b = 32 * (cs % 4) if COLTILE_S else 0
                            nc.tensor.matmul(Sacc[b:b + NK, :], ohs[cs],
                                             natB[:, cs, :],
                                             start=(cs < 4) if COLTILE_S else cs == 0,
                                             stop=(cs >= nchunk - 4) if COLTILE_S
                                             else cs == nchunk - 1,
                                             skip_group_check=True,
                                             tile_position=(0, b))
                            del ohs[cs]

                    # period-4 sub-batches: 4 chunks of transposes (short
                    # enough not to re-throttle the HAM), then 4 chunks of
                    # dense y+S matmuls (long enough to keep the PE warm)
                    ohq = {}
                    for k4 in range(0, nchunk + 12, 4):
                        for ci in range(k4, k4 + 4):
                            if ci < nchunk:
                                st_in(ci)
                        for ci in range(k4 - 4, k4):
                            if 0 <= ci < nchunk:
                                st_tr(ci)
                        for ci in range(k4 - 8, k4 - 4):
                            if 0 <= ci < nchunk:
                                ohq[ci] = st_y(ci)
                        cs = k4 - 12
                        if 0 <= cs < nchunk:
                            s_burst(cs, ohq)
                    # finalize inv10 = 10 / max(sqrt(SQ + CR + pbb), eps)
                    nc.vector.tensor_add(SQ, SQ, CR)
                    nrm = grp.tile([128, nchunk], F32, tag="nrm")
                    nc.scalar.activation(nrm, SQ, AF.Sqrt, bias=pbb_col)
                    nc.vector.tensor_scalar(nrm, nrm, EPS, 0.1,
                                            op0=ALU.max, op1=ALU.mult)
                    nc.vector.reciprocal(inv10, nrm)
                    centers = update_centers(Sacc)

                # ---------------- iterations 2..NUM_ITERS ----------------
                with tc.tile_pool(name="ps_it", bufs=2, space="PSUM") as ps_it:
                    ngrp = nchunk // GSZ
                    for it in range(NUM_ITERS - 1):
                        GB = itp.tile([128, 2, NK], BF16, tag="GB")
                        make_G(centers, lambda h: GB[:, h], want_h128=True)
                        Sacc = ps_acc.tile([128, C + 1], F32, tag="S")
                        prev_oh = None
                        for g in range(ngrp + 1):
                            ohg = None
                            if g < ngrp:
                                simg = ps_it.tile([128, GSZ, NK], F32, tag="sim")
                                for j in range(GSZ):
                                    ci = g * GSZ + j
                                    sl = slice(ci * 128, (ci + 1) * 128)
                                    nc.tensor.matmul(simg[:, j, :], fTB[:, 0, sl],
                                                     GB[:, 0], start=True, stop=False,
                                                     skip_group_check=True)
                                    nc.tensor.matmul(simg[:, j, :], fTB[:, 1, sl],
                                                     GB[:, 1], start=False, stop=True,
                                                     skip_group_check=True)
                                simh = grp.tile([128, GSZ, NK], F32, tag="simh")
                                nc.vector.tensor_tensor(simh, simg, h128g,
                                                        op=ALU.add)
                                mxg = grp.tile([128, GSZ], F32, tag="mxg")
                                nc.vector.tensor_reduce(
                                    mxg, simh, axis=mybir.AxisListType.X, op=ALU.max)
                                ohg = grp.tile([128, GSZ, NK], BF16, tag="ohg")
                                for j in range(GSZ):
                                    nc.vector.tensor_scalar(
                                        ohg[:, j, :], simh[:, j, :],
                                        mxg[:, j:j + 1], None, op0=ALU.is_ge)
                            if g > 0:
                                for j in range(GSZ):
                                    cs = (g - 1) * GSZ + j
                                    nc.tensor.matmul(
                                        Sacc[0:NK, :], prev_oh[:, j, :],
                                        natB[:, cs, :],
                                        start=(cs == 0), stop=(cs == nchunk - 1),
                                        skip_group_check=True,
                                        tile_position=(0, 0))
                            prev_oh = ohg
                        centers = update_centers(Sacc)

                    # ---------------- phase-5 prep ----------------
                    make_G(centers, lambda h: catB[:, h, C:C + NK], hrowB_dst=hFB)
                    # Dm = centers @ rwT + rb, replicated to partitions 0/32/64/96
                    cT = itp.tile([128, 2, NK], F32, tag="cT")
                    for h in range(2):
                        tp = ps_acc.tile([128, NK], F32, tag="ips")
                        nc.tensor.transpose(tp, centers[:, h * 128:(h + 1) * 128],
                                            ident[0:NK, 0:NK])
                        nc.vector.tensor_copy(cT[:, h], tp)
                    dm_ps = ps_acc.tile([NK, C], F32, tag="ips")
                    nc.tensor.matmul(dm_ps, cT[:, 0], rwT[:, 0],
                                     start=True, stop=False)
                    nc.tensor.matmul(dm_ps, cT[:, 1], rwT[:, 1],
                                     start=False, stop=False)
                    nc.tensor.matmul(dm_ps, ones_row[:, 0:NK], rb_row,
                                     start=False, stop=True)
                    DmB = itp.tile([NK, C], BF16, tag="DmB")
                    nc.vector.tensor_copy(DmB, dm_ps)
                    # replication matrix R^T [8, 4, 32]: R^T[k, j, r] = (r == k)
                    RrepB = res.tile([NK, 4, 32], BF16)
                    for q in range(4):
                        nc.vector.tensor_copy(RrepB[:, q, :], ident_b[0:NK, 0:32])
                    dmr_ps = ps_acc.tile([128, C], F32, tag="ips")
                    nc.tensor.matmul(dmr_ps, RrepB[:].rearrange("k a b -> k (a b)"),
                                     DmB)
                    nc.vector.tensor_copy(DmRepB, dmr_ps)

            # ---------------- phase 5: final weights + refine ----------------
            with (
                tc.tile_pool(name="ps_op", bufs=7, space="PSUM") as ps_op,
                tc.tile_pool(name="ps_wt", bufs=1, space="PSUM") as ps_wt,
            ):
                NG5 = nchunk // U_OUT
                st5 = {"optiles": {}, "wn": None, "wtsb": None}

                def p5_op(g, j0, j1):
                    # op_ matmuls for chunks j0..j1-1 of group g (+ their exps)
                    for j in range(j0, j1):
                        ci = g * U_OUT + j
                        sl = slice(ci * 128, (ci + 1) * 128)
                        op_ = ps_op.tile([128, C + NK], F32, tag="op", name="op_")
                        nc.tensor.matmul(op_, fTB[:, 0, sl], catB[:, 0],
                                         start=True, stop=False)
                        nc.tensor.matmul(op_[:, C:C + NK], ones_rowB, hFB,
                                         start=False, stop=False,
                                         skip_group_check=True)
                        nc.tensor.matmul(op_, fTB[:, 1, sl], catB[:, 1],
                                         start=False, stop=False,
                                         skip_group_check=True)
                        st5["optiles"][ci] = op_

                def p5_exps(g):
                    st5["esg"] = grp.tile([128, U_OUT, NK], BF16,
                                          tag="esg", name="esg")
                    for j in range(U_OUT):
                        ci = g * U_OUT + j
                        nc.scalar.activation(st5["esg"][:, j, :],
                                             st5["optiles"][ci][:, C:C + NK],
                                             AF.Exp, scale=inv10[:, ci:ci + 1])

                def p5_soft(g, esg):
                    # den + normalized weights into the padded wn tile
                    den4 = sml.tile([128, U_OUT], F32, tag="den4")
                    nc.vector.tensor_reduce(den4, esg, axis=mybir.AxisListType.X,
                                            op=ALU.add)
                    rd = sml.tile([128, U_OUT], F32, tag="rd")
                    nc.vector.reciprocal(rd, den4)
                    wn = grp.tile([128, U_OUT, 32], BF16, tag="wn", name="wn")
                    nc.vector.memset(wn[:, :, NK:32], 0.0)
                    for j in range(U_OUT):
                        nc.vector.tensor_scalar_mul(wn[:, j, 0:NK], esg[:, j, :],
                                                    rd[:, j:j + 1])
                    st5["wn"] = wn

                def p5_tr(g):
                    wt_ps = ps_wt.tile([128, 128], BF16, tag="wt")
                    nc.tensor.transpose(
                        wt_ps, st5["wn"][:].rearrange("p a b -> p (a b)"), ident_b)
                    wtsb = sml.tile([128, 128], BF16, tag="wtsb")
                    nc.vector.tensor_copy(wtsb, wt_ps)
                    st5["wtsb"] = wtsb

                def p5_w(g):
                    # 4 row-tiled concurrent w@Dm accumulations + output copies
                    wtsb = st5["wtsb"]
                    for j in range(U_OUT):
                        ci = g * U_OUT + j
                        op_ = st5["optiles"][ci]
                        b = 32 * j
                        nc.tensor.matmul(op_[:, 0:C], wtsb[b:b + NK, :],
                                         DmRepB[b:b + NK, :],
                                         start=False, stop=True,
                                         skip_group_check=True,
                                         tile_position=(b, 0))
                    ot = outp.tile([128, U_OUT, C], F32, tag="ot", name="ot")
                    for j in range(U_OUT):
                        ci = g * U_OUT + j
                        op_ = st5["optiles"].pop(ci)
                        nc.scalar.activation(ot[:, j, 0:128], op_[:, 0:128], AF.Copy)
                        nc.vector.tensor_copy(ot[:, j, 128:C], op_[:, 128:C])
                    nc.sync.dma_start(out=out_v[:, g * U_OUT:(g + 1) * U_OUT, :],
                                      in_=ot)

                prev_esg = None
                for g in range(NG5 + 1):
                    if 1 <= g:
                        p5_soft(g - 1, prev_esg)
                    if g < NG5:
                        p5_op(g, 0, 1)
                    if 1 <= g:
                        p5_tr(g - 1)
                    if g < NG5:
                        p5_op(g, 1, 2)
                    if 1 <= g:
                        p5_w(g - 1)
                    if g < NG5:
                        p5_op(g, 2, U_OUT)
                        p5_exps(g)
                    prev_esg = st5.get("esg")

    nc.compile()
    return nc


_NC = None
TRACE = False
TRACE_DIR = None
LAST_EXEC_NS = None


def kernel(F_p, proj_w, proj_b, refine_w, refine_b):
    global _NC, LAST_EXEC_NS
    if _NC is None:
        _NC = build_bass()
    F_p = np.ascontiguousarray(F_p, dtype=np.float32)
    shared = {
        "pw": np.ascontiguousarray(proj_w, dtype=np.float32),
        "pb": np.ascontiguousarray(proj_b, dtype=np.float32),
        "rw": np.ascontiguousarray(refine_w, dtype=np.float32),
        "rb": np.ascontiguousarray(refine_b, dtype=np.float32),
    }
    in_maps = [{"fp": F_p[i], **shared} for i in range(N_CORES)]
    res = run_bass_kernel_spmd(_NC, in_maps, list(range(N_CORES)), trace=TRACE,
                               tmpdir=TRACE_DIR)
    LAST_EXEC_NS = res.exec_time_ns
    return np.stack([res.results[i]["out"] for i in range(N_CORES)], axis=0)


# revision 22
# speedup vs baseline: 1.0156x; 1.0156x over previous
"""NodeClustering (vq_codebook) Trainium2 kernel — v3.

Math (per batch element b, P=16384 points, C=256 channels, K=8 clusters):
  nodes = F_p @ proj_w.T + proj_b
  3 iterations of: sim = l2(nodes) @ l2(centers).T ; assign = argmax;
                   centers = segment_mean(nodes)
  weights = softmax(10 * l2(nodes) @ l2(centers).T)
  out = (weights@centers + F_p) @ refine_w.T + refine_b

Algebraic restructuring (as v1):
  nodes[p] . cn_k    = F_p[p] . G_k + h_k      (G = A @ cn, h = pb . cn)
  segment_sum(nodes) = segment_sum(F_p) @ pwT + counts * pb
  out                = F_p @ rwT + w @ Dm      (Dm = centers @ rwT + rb)
  ||nodes_p||^2      = ||F_p@pwT||^2 + F_p.(2 pwT@pb) + ||pb||^2
                       (cross term rides as an extra rhs column)

Execution notes (measured: PE runs ~1.2 GHz, per-matmul cost ~(91+N)/1.2 ns,
so instruction count and N both matter):
  - fT via PE transposes (not Sync DMA transposes), batched PSUM->SBUF copy.
  - Software-pipelined chunk skew so the in-order PE queue never waits on
    DVE/ACT results.
  - S-matmuls col-tiled 4x: four chunks' onehot weights live in distinct
    32-column strips of the PE array and stream concurrently; partials at
    partition 0/32/64/96 are combined by 4 diagonal-identity matmuls.
  - Iteration argmax batched 8 chunks per DVE op; +h via broadcast DVE add.
  - Phase 5: per-4-chunk padded weight transpose (one PE op), row-tiled 4x
    concurrent w@Dm accumulating into each chunk's op_ PSUM (weights
    prenormalized), DmB replicated to partitions 0/32/64/96.

Sharding: pure data parallel, core i <- batch element i (B=8, 8 cores).
"""

import sys
import numpy as np

sys.path.insert(0, "/opt/trn_rl_repo")

import concourse.bass as bass
import concourse.bacc as bacc
import concourse.mybir as mybir
import concourse.tile as tile
from concourse._compat import get_trn_type
from concourse.bass_utils import axon_active
from concourse.masks import make_identity
from concourse.bass_utils import run_bass_kernel_spmd

P = 16384
C = 256
NK = 8
NUM_ITERS = 3
EPS = 1e-12
N_CORES = 8
NCHUNK = P // 128
U_IN = 8    # chunks per input DMA (1 MiB)
U_OUT = 4   # chunks per output DMA / phase-5 group
GSZ = 8     # chunks per argmax batch in iterations
SKW_T = 2   # transpose lag behind cast (phase 1)
SKW_Y = 6   # y matmul lag behind transpose copy (phase 1)
SKW_S = 10  # S matmul lag behind argmax (phase 1)

F32 = mybir.dt.float32
BF16 = mybir.dt.bfloat16
AF = mybir.ActivationFunctionType
ALU = mybir.AluOpType

COLTILE_S = False   # bisect: col-tiled S matmuls
ROWTILE_W = True    # bisect: row-tiled phase-5 wDm


def build_bass(p=P):
    nchunk = p // 128
    idx = list(np.linspace(0, p - 1, NK).astype(np.int64))
    nc = bacc.Bacc(
        get_trn_type() or "TRN2",
        target_bir_lowering=False,
        debug=not axon_active(),
        num_devices=N_CORES,
    )

    fp = nc.dram_tensor("fp", [p, C], F32, kind="ExternalInput")
    pw = nc.dram_tensor("pw", [C, C], F32, kind="ExternalInput")
    pb = nc.dram_tensor("pb", [C], F32, kind="ExternalInput")
    rw = nc.dram_tensor("rw", [C, C], F32, kind="ExternalInput")
    rb = nc.dram_tensor("rb", [C], F32, kind="ExternalInput")
    out = nc.dram_tensor("out", [p, C], F32, kind="ExternalOutput")

    fp_v = fp[:].rearrange("(n p) c -> p n c", p=128)
    out_v = out[:].rearrange("(n p) c -> p n c", p=128)

    with tile.TileContext(nc) as tc:
        with (
            tc.tile_pool(name="res", bufs=1) as res,        # persistent tiles
            tc.tile_pool(name="nat", bufs=3) as natp,       # streamed F_p chunks
            tc.tile_pool(name="outp", bufs=3) as outp,      # output staging
            tc.tile_pool(name="sml", bufs=12) as sml,       # per-chunk small tiles
            tc.tile_pool(name="grp", bufs=2) as grp,        # group-batched tiles
            tc.tile_pool(name="it", bufs=1) as itp,         # per-iteration small tiles
        ):
            # ---------------- constants + weights ----------------
            ident = res.tile([128, 128], F32)
            make_identity(nc, ident)
            ident_b = res.tile([128, 128], BF16)
            make_identity(nc, ident_b)
            ones_row = res.tile([1, 128], F32)
            nc.vector.memset(ones_row, 1.0)
            ones_rowB = res.tile([1, 128], BF16)
            nc.vector.memset(ones_rowB, 1.0)

            pw_n = res.tile([128, 2, C], F32)    # proj_w rows (c' partition)
            nc.sync.dma_start(out=pw_n, in_=pw[:].rearrange("(h p) c -> p h c", p=128))
            rw_n = res.tile([128, 2, C], F32)
            nc.sync.dma_start(out=rw_n, in_=rw[:].rearrange("(h p) c -> p h c", p=128))
            pb_col = res.tile([128, 2], F32)
            nc.sync.dma_start(out=pb_col, in_=pb[:].rearrange("(h p) -> p h", p=128))
            pb_row = res.tile([1, C], F32)
            nc.sync.dma_start(out=pb_row, in_=pb[:].unsqueeze(0))
            rb_row = res.tile([1, C], F32)
            nc.sync.dma_start(out=rb_row, in_=rb[:].unsqueeze(0))

            # residents
            natB = res.tile([128, nchunk, C + 1], BF16)  # F_p natural + ones col
            nc.vector.memset(natB[:, :, C:C + 1], 1.0)
            fTB = res.tile([128, 2, p], BF16)   # F_p.T halves (c partition)
            SQ = res.tile([128, nchunk], F32)   # ||F@pwT||^2 per point
            CR = res.tile([128, nchunk], F32)   # cross term per point
            inv10 = res.tile([128, nchunk], F32)
            pwT = res.tile([128, 2, C], F32)
            rwT = res.tile([128, 2, C], F32)
            # [pwT | v2 | G1] bf16 rhs for the phase-1 matmul
            projrhsB = res.tile([128, 2, C + 1 + NK], BF16)
            hrow1B = res.tile([1, NK], BF16)    # iter-1 h row (ones matmul rhs)
            # [rwT | Gf] bf16 rhs for phase 5
            catB = res.tile([128, 2, C + NK], BF16)
            hFB = res.tile([1, NK], BF16)       # ones-row rhs for phase-5 sims
            DmRepB = res.tile([128, C], BF16)   # Dm replicated at 0/32/64/96
            h128 = res.tile([128, NK], F32)     # iteration h, partition-bcast
            h128g = res.tile([128, GSZ, NK], F32)  # h replicated per group slot
            pbb_col = res.tile([128, 1], F32)   # ||pb||^2 broadcast

            with tc.tile_pool(name="ps_acc", bufs=1, space="PSUM") as ps_acc:

                def make_G(centers_sb, Gdst, hrowB_dst=None, want_h128=False):
                    """centers (8,C) f32 -> l2norm -> bf16 G halves + h outputs."""
                    csq = itp.tile([NK, C], F32, tag="csq")
                    cn2 = itp.tile([NK, 1], F32, tag="cn2")
                    nc.scalar.activation(csq, centers_sb, AF.Square, accum_out=cn2)
                    nc.scalar.sqrt(cn2, cn2)
                    nc.vector.tensor_scalar(cn2, cn2, EPS, None, op0=ALU.max)
                    rin = itp.tile([NK, 1], F32, tag="rin")
                    nc.vector.reciprocal(rin, cn2)
                    cn = itp.tile([NK, C], F32, tag="cn")
                    nc.vector.tensor_scalar_mul(cn, centers_sb, rin)
                    cnT = itp.tile([128, 2, NK], F32, tag="cnT")
                    for h in range(2):
                        tp = ps_acc.tile([128, NK], F32, tag="ips")
                        nc.tensor.transpose(tp, cn[:, h * 128:(h + 1) * 128],
                                            ident[0:NK, 0:NK])
                        nc.vector.tensor_copy(cnT[:, h], tp)
                    # G[c,k] = sum_c' proj_w[c',c] cnT[c',k]
                    for mh in range(2):
                        gp = ps_acc.tile([128, NK], F32, tag="ips")
                        nc.tensor.matmul(gp, pw_n[:, 0, mh * 128:(mh + 1) * 128],
                                         cnT[:, 0], start=True, stop=False)
                        nc.tensor.matmul(gp, pw_n[:, 1, mh * 128:(mh + 1) * 128],
                                         cnT[:, 1], start=False, stop=True)
                        nc.vector.tensor_copy(Gdst(mh), gp)
                    # h_k = pb . cn_k
                    hp = ps_acc.tile([1, NK], F32, tag="ips2")
                    nc.tensor.matmul(hp, pb_col[:, 0:1], cnT[:, 0],
                                     start=True, stop=False)
                    nc.tensor.matmul(hp, pb_col[:, 1:2], cnT[:, 1],
                                     start=False, stop=True)
                    if hrowB_dst is not None:
                        nc.vector.tensor_copy(hrowB_dst, hp)
                    if want_h128:
                        hrow_sb = itp.tile([1, NK], F32, tag="hrow")
                        nc.vector.tensor_copy(hrow_sb, hp)
                        hbp = ps_acc.tile([128, NK], F32, tag="ips2")
                        nc.tensor.matmul(hbp, ones_row, hrow_sb)
                        nc.vector.tensor_copy(h128, hbp)
                        for jj in range(GSZ):
                            nc.vector.tensor_copy(h128g[:, jj, :], h128)

                def combine_S(Sacc, nstrip):
                    """Sum the col-tiled partials (partitions 0/32/..)."""
                    if nstrip == 1:
                        return Sacc[0:NK, :]
                    Ssb = itp.tile([64, C + 1], F32, tag="Ssb")
                    nc.vector.tensor_copy(Ssb, Sacc[0:64, :])
                    S2 = ps_acc.tile([NK, C + 1], F32, tag="ips2")
                    for q in range(nstrip):
                        b = 32 * q
                        nc.tensor.matmul(S2, ident[b:b + NK, b:b + NK],
                                         Ssb[b:b + NK, :],
                                         start=(q == 0), stop=(q == nstrip - 1),
                                         tile_position=(b, 0))
                    return S2

                def update_centers(Sacc, nstrip=1):
                    S2 = combine_S(Sacc, nstrip)
                    cnt = itp.tile([NK, 1], F32, tag="cntsb")
                    nc.vector.tensor_scalar(cnt, S2[:, C:C + 1], 1.0, None,
                                            op0=ALU.max)
                    nc.vector.reciprocal(cnt, cnt)
                    fmean = itp.tile([NK, C], F32, tag="fmean")
                    nc.vector.tensor_scalar_mul(fmean, S2[:, 0:C], cnt)
                    fmT = itp.tile([128, 2, NK], F32, tag="fmT")
                    for h in range(2):
                        tp = ps_acc.tile([128, NK], F32, tag="ips")
                        nc.tensor.transpose(tp, fmean[:, h * 128:(h + 1) * 128],
                                            ident[0:NK, 0:NK])
                        nc.vector.tensor_copy(fmT[:, h], tp)
                    cp = ps_acc.tile([NK, C], F32, tag="ips")
                    nc.tensor.matmul(cp, fmT[:, 0], pwT[:, 0], start=True, stop=False)
                    nc.tensor.matmul(cp, fmT[:, 1], pwT[:, 1], start=False, stop=False)
                    nc.tensor.matmul(cp, ones_row[:, 0:NK], pb_row,
                                     start=False, stop=True)
                    centers = itp.tile([NK, C], F32, tag="centers")
                    nc.vector.tensor_copy(centers, cp)
                    return centers

                # ---------------- phase 1 (+ setup) ----------------
                with (
                    tc.tile_pool(name="ps_tr", bufs=2, space="PSUM") as ps_tr,
                    tc.tile_pool(name="ps_y", bufs=3, space="PSUM") as ps_y,
                ):
                    # pwT/rwT: transposed weights via PE (f32)
                    for dst, src in ((pwT, pw_n), (rwT, rw_n)):
                        for kh in range(2):      # source partition half (c')
                            for mh in range(2):  # source free half (c)
                                tp = ps_acc.tile(
                                    [128, 128], F32,
                                    tag="ips" if (kh + mh) % 2 == 0 else "ips2")
                                nc.tensor.transpose(
                                    tp, src[:, kh, mh * 128:(mh + 1) * 128], ident)
                                if (kh + mh) % 2:
                                    nc.vector.tensor_copy(
                                        dst[:, mh, kh * 128:(kh + 1) * 128], tp)
                                else:
                                    nc.scalar.activation(
                                        dst[:, mh, kh * 128:(kh + 1) * 128], tp,
                                        AF.Copy)
                    for h in range(2):
                        nc.vector.tensor_copy(projrhsB[:, h, 0:C], pwT[:, h])
                        nc.scalar.activation(catB[:, h, 0:C], rwT[:, h], AF.Copy)
                    # v2 = 2 * pwT @ pb, split into column halves
                    for mh in range(2):
                        vp = ps_acc.tile([128, 1], F32, tag="ips")
                        nc.tensor.matmul(vp, pw_n[:, 0, mh * 128:(mh + 1) * 128],
                                         pb_col[:, 0:1], start=True, stop=False)
                        nc.tensor.matmul(vp, pw_n[:, 1, mh * 128:(mh + 1) * 128],
                                         pb_col[:, 1:2], start=False, stop=True)
                        nc.scalar.activation(projrhsB[:, mh, C:C + 1], vp, AF.Copy,
                                             scale=2.0)
                    # pbb = ||pb||^2 broadcast to a [128,1] column
                    pbsq = itp.tile([1, C], F32, tag="pbsq")
                    pbb1 = itp.tile([1, 1], F32, tag="pbb1")
                    nc.scalar.activation(pbsq, pb_row, AF.Square, accum_out=pbb1)
                    pbb_ps = ps_acc.tile([128, 1], F32, tag="ips")
                    nc.tensor.matmul(pbb_ps, ones_row, pbb1)
                    nc.vector.tensor_copy(pbb_col, pbb_ps)

                    # seed rows of F_p for initial centers, gathered from DRAM
                    gatT = itp.tile([NK, C], F32, tag="gatT")
                    for k, g in enumerate(idx):
                        nc.sync.dma_start(out=gatT[k:k + 1, :], in_=fp[g:g + 1, :])
                    gatc = itp.tile([128, 2, NK], F32, tag="gatc")
                    for h in range(2):
                        tp = ps_acc.tile([128, NK], F32, tag="ips")
                        nc.tensor.transpose(tp, gatT[:, h * 128:(h + 1) * 128],
                                            ident[0:NK, 0:NK])
                        nc.vector.tensor_copy(gatc[:, h], tp)
                    c0 = ps_acc.tile([NK, C], F32, tag="ips")
                    nc.tensor.matmul(c0, gatc[:, 0], pwT[:, 0], start=True, stop=False)
                    nc.tensor.matmul(c0, gatc[:, 1], pwT[:, 1], start=False, stop=False)
                    nc.tensor.matmul(c0, ones_row[:, 0:NK], pb_row,
                                     start=False, stop=True)
                    centers = itp.tile([NK, C], F32, tag="centers")
                    nc.vector.tensor_copy(centers, c0)
                    make_G(centers, lambda h: projrhsB[:, h, C + 1:C + 1 + NK],
                           hrowB_dst=hrow1B)

                    # ---- streamed loop ----
                    Sacc = ps_acc.tile([128, C + 1], F32, tag="S")
                    state = {"nt": None,
                             "sqt": grp.tile([128, GSZ, C], BF16, tag="sqt",
                                             name="sqt")}

                    def st_in(ci):
                        if ci % U_IN == 0:
                            state["nt"] = natp.tile([128, U_IN, C], F32, tag="nat",
                                                    name="nt")
                            gi = ci // U_IN
                            nc.sync.dma_start(
                                out=state["nt"],
                                in_=fp_v[:, gi * U_IN:(gi + 1) * U_IN, :])
                        j = ci % U_IN
                        if ci % 4 == 1:
                            nc.vector.tensor_copy(natB[:, ci, 0:C],
                                                  state["nt"][:, j, :])
                        else:
                            nc.gpsimd.tensor_copy(natB[:, ci, 0:C],
                                                  state["nt"][:, j, :])

                    def st_tr(ci):
                        sl = slice(ci * 128, (ci + 1) * 128)
                        trt = ps_tr.tile([128, 2, 128], BF16, tag="tr")
                        nc.tensor.transpose(trt[:, 0, :], natB[:, ci, 0:128],
                                            ident_b)
                        nc.tensor.transpose(trt[:, 1, :], natB[:, ci, 128:C],
                                            ident_b)
                        nc.scalar.activation(fTB[:, :, sl], trt, AF.Copy)

                    def st_y(ci):
                        sl = slice(ci * 128, (ci + 1) * 128)
                        y = ps_y.tile([128, C + 1 + NK], F32, tag="y")
                        nc.tensor.matmul(y, fTB[:, 0, sl], projrhsB[:, 0],
                                         start=True, stop=False)
                        nc.tensor.matmul(y[:, C + 1:C + 1 + NK], ones_rowB, hrow1B,
                                         start=False, stop=False,
                                         skip_group_check=True)
                        nc.tensor.matmul(y, fTB[:, 1, sl], projrhsB[:, 1],
                                         start=False, stop=True)
                        nc.scalar.activation(state["sqt"][:, ci % GSZ, :],
                                             y[:, 0:C], AF.Square)
                        nc.vector.tensor_copy(CR[:, ci:ci + 1], y[:, C:C + 1])
                        mx = sml.tile([128, 1], F32, tag="mx")
                        nc.vector.tensor_reduce(mx, y[:, C + 1:C + 1 + NK],
                                                axis=mybir.AxisListType.X, op=ALU.max)
                        oh = sml.tile([128, NK], BF16, tag="oh")
                        nc.vector.tensor_scalar(oh, y[:, C + 1:C + 1 + NK],
                                                mx[:, 0:1], None, op0=ALU.is_ge)
                        if ci % GSZ == GSZ - 1:
                            g0 = (ci // GSZ) * GSZ
                            nc.vector.tensor_reduce(
                                SQ[:, g0:g0 + GSZ], state["sqt"],
                                axis=mybir.AxisListType.X, op=ALU.add)
                            state["sqt"] = grp.tile([128, GSZ, C], BF16, tag="sqt",
                                                    name="sqt")
                        return oh

                    def s_burst(c0, ohs):
                        # 4 col-tiled S matmuls: chunk c0+q -> col strip q
                        for q in range(4):
                            cs = c0 + q
                            b = 32 * (cs % 4) if COLTILE_S else 0
                            nc.tensor.matmul(Sacc[b:b + NK, :], ohs[cs],
                                             natB[:, cs, :],
                                             start=(cs < 4) if COLTILE_S else cs == 0,
                                             stop=(cs >= nchunk - 4) if COLTILE_S
                                             else cs == nchunk - 1,
                                             skip_group_check=True,
                                             tile_position=(0, b))
                            del ohs[cs]

                    ohq = {}
                    for ci in range(nchunk + SKW_S):
                        if ci < nchunk:
                            st_in(ci)
                        cy = ci - SKW_Y
                        if 0 <= cy < nchunk:
                            ohq[cy] = st_y(cy)
                        ct = ci - SKW_T
                        if 0 <= ct < nchunk:
                            st_tr(ct)
                        cs = ci - SKW_S
                        if 0 <= cs < nchunk and cs % 4 == 3:
                            s_burst(cs - 3, ohq)
                    # finalize inv10 = 10 / max(sqrt(SQ + CR + pbb), eps)
                    nc.vector.tensor_add(SQ, SQ, CR)
                    nrm = grp.tile([128, nchunk], F32, tag="nrm")
                    nc.scalar.activation(nrm, SQ, AF.Sqrt, bias=pbb_col)
                    nc.vector.tensor_scalar(nrm, nrm, EPS, 0.1,
                                            op0=ALU.max, op1=ALU.mult)
                    nc.vector.reciprocal(inv10, nrm)
                    centers = update_centers(Sacc)

                # ---------------- iterations 2..NUM_ITERS ----------------
                with tc.tile_pool(name="ps_it", bufs=2, space="PSUM") as ps_it:
                    ngrp = nchunk // GSZ
                    for it in range(NUM_ITERS - 1):
                        GB = itp.tile([128, 2, NK], BF16, tag="GB")
                        make_G(centers, lambda h: GB[:, h], want_h128=True)
                        Sacc = ps_acc.tile([128, C + 1], F32, tag="S")
                        prev_oh = None
                        for g in range(ngrp + 1):
                            ohg = None
                            if g < ngrp:
                                simg = ps_it.tile([128, GSZ, NK], F32, tag="sim")
                                for j in range(GSZ):
                                    ci = g * GSZ + j
                                    sl = slice(ci * 128, (ci + 1) * 128)
                                    nc.tensor.matmul(simg[:, j, :], fTB[:, 0, sl],
                                                     GB[:, 0], start=True, stop=False,
                                                     skip_group_check=True)
                                    nc.tensor.matmul(simg[:, j, :], fTB[:, 1, sl],
                                                     GB[:, 1], start=False, stop=True,
                                                     skip_group_check=True)
                                simh = grp.tile([128, GSZ, NK], F32, tag="simh")
                                nc.vector.tensor_tensor(simh, simg, h128g,
                                                        op=ALU.add)
                                mxg = grp.tile([128, GSZ], F32, tag="mxg")
                                nc.vector.tensor_reduce(
                                    mxg, simh, axis=mybir.AxisListType.X, op=ALU.max)
                                ohg = grp.tile([128, GSZ, NK], BF16, tag="ohg")
                                for j in range(GSZ):
                                    nc.vector.tensor_scalar(
                                        ohg[:, j, :], simh[:, j, :],
                                        mxg[:, j:j + 1], None, op0=ALU.is_ge)
                            if g > 0:
                                for j in range(GSZ):
                                    cs = (g - 1) * GSZ + j
                                    nc.tensor.matmul(
                                        Sacc[0:NK, :], prev_oh[:, j, :],
                                        natB[:, cs, :],
                                        start=(cs == 0), stop=(cs == nchunk - 1),
                                        skip_group_check=True,
                                        tile_position=(0, 0))
                            prev_oh = ohg
                        centers = update_centers(Sacc)

                    # ---------------- phase-5 prep ----------------
                    make_G(centers, lambda h: catB[:, h, C:C + NK], hrowB_dst=hFB)
                    # Dm = centers @ rwT + rb, replicated to partitions 0/32/64/96
                    cT = itp.tile([128, 2, NK], F32, tag="cT")
                    for h in range(2):
                        tp = ps_acc.tile([128, NK], F32, tag="ips")
                        nc.tensor.transpose(tp, centers[:, h * 128:(h + 1) * 128],
                                            ident[0:NK, 0:NK])
                        nc.vector.tensor_copy(cT[:, h], tp)
                    dm_ps = ps_acc.tile([NK, C], F32, tag="ips")
                    nc.tensor.matmul(dm_ps, cT[:, 0], rwT[:, 0],
                                     start=True, stop=False)
                    nc.tensor.matmul(dm_ps, cT[:, 1], rwT[:, 1],
                                     start=False, stop=False)
                    nc.tensor.matmul(dm_ps, ones_row[:, 0:NK], rb_row,
                                     start=False, stop=True)
                    DmB = itp.tile([NK, C], BF16, tag="DmB")
                    nc.vector.tensor_copy(DmB, dm_ps)
                    # replication matrix R^T [8, 4, 32]: R^T[k, j, r] = (r == k)
                    RrepB = res.tile([NK, 4, 32], BF16)
                    for q in range(4):
                        nc.vector.tensor_copy(RrepB[:, q, :], ident_b[0:NK, 0:32])
                    dmr_ps = ps_acc.tile([128, C], F32, tag="ips")
                    nc.tensor.matmul(dmr_ps, RrepB[:].rearrange("k a b -> k (a b)"),
                                     DmB)
                    nc.vector.tensor_copy(DmRepB, dmr_ps)

            # ---------------- phase 5: final weights + refine ----------------
            with (
                tc.tile_pool(name="ps_op", bufs=7, space="PSUM") as ps_op,
                tc.tile_pool(name="ps_wt", bufs=1, space="PSUM") as ps_wt,
            ):
                NG5 = nchunk // U_OUT
                st5 = {"optiles": {}, "wn": None, "wtsb": None}

                def p5_op(g, j0, j1):
                    # op_ matmuls for chunks j0..j1-1 of group g (+ their exps)
                    for j in range(j0, j1):
                        ci = g * U_OUT + j
                        sl = slice(ci * 128, (ci + 1) * 128)
                        op_ = ps_op.tile([128, C + NK], F32, tag="op", name="op_")
                        nc.tensor.matmul(op_, fTB[:, 0, sl], catB[:, 0],
                                         start=True, stop=False)
                        nc.tensor.matmul(op_[:, C:C + NK], ones_rowB, hFB,
                                         start=False, stop=False,
                                         skip_group_check=True)
                        nc.tensor.matmul(op_, fTB[:, 1, sl], catB[:, 1],
                                         start=False, stop=False,
                                         skip_group_check=True)
                        st5["optiles"][ci] = op_

                def p5_exps(g):
                    st5["esg"] = grp.tile([128, U_OUT, NK], BF16,
                                          tag="esg", name="esg")
                    for j in range(U_OUT):
                        ci = g * U_OUT + j
                        nc.scalar.activation(st5["esg"][:, j, :],
                                             st5["optiles"][ci][:, C:C + NK],
                                             AF.Exp, scale=inv10[:, ci:ci + 1])

                def p5_soft(g, esg):
                    # den + normalized weights into the padded wn tile
                    den4 = sml.tile([128, U_OUT], F32, tag="den4")
                    nc.vector.tensor_reduce(den4, esg, axis=mybir.AxisListType.X,
                                            op=ALU.add)
                    rd = sml.tile([128, U_OUT], F32, tag="rd")
                    nc.vector.reciprocal(rd, den4)
                    wn = grp.tile([128, U_OUT, 32], BF16, tag="wn", name="wn")
                    nc.vector.memset(wn[:, :, NK:32], 0.0)
                    for j in range(U_OUT):
                        nc.vector.tensor_scalar_mul(wn[:, j, 0:NK], esg[:, j, :],
                                                    rd[:, j:j + 1])
                    st5["wn"] = wn

                def p5_tr(g):
                    wt_ps = ps_wt.tile([128, 128], BF16, tag="wt")
                    nc.tensor.transpose(
                        wt_ps, st5["wn"][:].rearrange("p a b -> p (a b)"), ident_b)
                    wtsb = sml.tile([128, 128], BF16, tag="wtsb")
                    nc.vector.tensor_copy(wtsb, wt_ps)
                    st5["wtsb"] = wtsb

                def p5_w(g):
                    # 4 row-tiled concurrent w@Dm accumulations + output copies
                    wtsb = st5["wtsb"]
                    for j in range(U_OUT):
                        ci = g * U_OUT + j
                        op_ = st5["optiles"][ci]
                        b = 32 * j
                        nc.tensor.matmul(op_[:, 0:C], wtsb[b:b + NK, :],
                                         DmRepB[b:b + NK, :],
                                         start=False, stop=True,
                                         skip_group_check=True,
                                         tile_position=(b, 0))
                    ot = outp.tile([128, U_OUT, C], F32, tag="ot", name="ot")
                    for j in range(U_OUT):
                        ci = g * U_OUT + j
                        op_ = st5["optiles"].pop(ci)
                        nc.scalar.activation(ot[:, j, 0:128], op_[:, 0:128], AF.Copy)
                        nc.vector.tensor_copy(ot[:, j, 128:C], op_[:, 128:C])
                    nc.sync.dma_start(out=out_v[:, g * U_OUT:(g + 1) * U_OUT, :],
                                      in_=ot)

                prev_esg = None
                for g in range(NG5 + 1):
                    if 1 <= g:
                        p5_soft(g - 1, prev_esg)
                    if g < NG5:
                        p5_op(g, 0, 1)
                    if 1 <= g:
                        p5_tr(g - 1)
                    if g < NG5:
                        p5_op(g, 1, 2)
                    if 1 <= g:
                        p5_w(g - 1)
                    if g < NG5:
                        p5_op(g, 2, U_OUT)
                        p5_exps(g)
                    prev_esg = st5.get("esg")

    nc.compile()
    return nc


_NC = None
TRACE = False
TRACE_DIR = None
LAST_EXEC_NS = None


def kernel(F_p, proj_w, proj_b, refine_w, refine_b):
    global _NC, LAST_EXEC_NS
    if _NC is None:
        _NC = build_bass()
    F_p = np.ascontiguousarray(F_p, dtype=np.float32)
    shared = {
        "pw": np.ascontiguousarray(proj_w, dtype=np.float32),
        "pb": np.ascontiguousarray(proj_b, dtype=np.float32),
        "rw": np.ascontiguousarray(refine_w, dtype=np.float32),
        "rb": np.ascontiguousarray(refine_b, dtype=np.float32),
    }
    in_maps = [{"fp": F_p[i], **shared} for i in range(N_CORES)]
    res = run_bass_kernel_spmd(_NC, in_maps, list(range(N_CORES)), trace=TRACE,
                               tmpdir=TRACE_DIR)
    LAST_EXEC_NS = res.exec_time_ns
    return np.stack([res.results[i]["out"] for i in range(N_CORES)], axis=0)
